# revision 11
# baseline (speedup 1.0000x reference)
"""Causal GQA self-attention (B=4, T=2048, D=2048, H=16, Hkv=4, RoPE) on 8 TRN2
NeuronCores.

Sharding: core = (batch b, stripe h) with b = core//2, h = core%2. Query rows of
each batch are interleaved in 128-row strips: stripe h owns global strips
{2s+h : s in 0..7} (1024 rows). Causal work is balanced across the two stripes
and the output rows are disjoint, so there are no collectives — the host
scatters the 8 [1024, 2048] results back into [4, 2048, 2048].

PSUM is managed as four 2-bank tiles ([128, 2, 512] f32). Phase A runs as two
passes (K then V) so only 4 banks accumulate per tb and evacuation of tb p
overlaps accumulation of tb p+1 (bank parity). Attention scores for two
128-key chunks land in one 2-bank tile so a single ACT exp call covers both
(amortizing the ~352-cycle ACT startup); causal masks are preloaded into PSUM
with an identity matmul (start=True sets has_written only on the masked
columns, the score matmul then accumulates there and overwrites elsewhere),
keeping masking off the DVE. Probability row-sum partials (dacc) accumulate in
bf16 on the DVE (2x mode); the per-query denominator is a ones-stationary
matmul over dacc at pair end, reciprocal on DVE, broadcast across partitions
with an outer-product matmul. RoPE uses partition-shifted DMA copies (sign
folded into the bf16 sin table); its second multiply runs on gpsimd to keep
the DVE clear.

Per-core asymmetry (stripe masks, RoPE tables at the stripe's global rows, the
gathered xT columns) is shipped as input data so the SPMD program is identical
on every core.
"""

import numpy as np

import concourse.bass as bass
import concourse.tile as tile
from concourse import bacc, mybir
from concourse.bass_utils import run_bass_kernel_spmd

F32 = mybir.dt.float32
F32R = mybir.dt.float32r
BF16 = mybir.dt.bfloat16
AF = mybir.ActivationFunctionType

B, T, D = 4, 2048, 2048
H, HKV, DH = 16, 4, 128
P = 128
NC_COUNT = 8
QL = 1024            # local query rows per core
NCH = D // P         # 16 contraction chunks
ROPE_BASE = 10000.0
NEG = -1.0e9

_CACHE = {}


def _build():
    nc = bacc.Bacc("TRN2", target_bir_lowering=False, debug=False,
                   num_devices=NC_COUNT)

    xT = nc.declare_dram_parameter("xT", [D, T // 2], BF16, isOutput=False)
    xTq = nc.declare_dram_parameter("xTq", [D, QL], BF16, isOutput=False)
    wq = nc.declare_dram_parameter("wq", [D, H * DH], BF16, isOutput=False)
    wkv = nc.declare_dram_parameter("wkv", [D, 2 * HKV * DH], BF16, isOutput=False)
    wo = nc.declare_dram_parameter("wo", [D, D], BF16, isOutput=False)
    cosq = nc.declare_dram_parameter("cosq", [DH, QL], F32, isOutput=False)
    sinq = nc.declare_dram_parameter("sinq", [DH, QL], BF16, isOutput=False)
    cosk = nc.declare_dram_parameter("cosk", [DH, T // 2], F32, isOutput=False)
    sink = nc.declare_dram_parameter("sink", [DH, T // 2], BF16, isOutput=False)
    qmask = nc.declare_dram_parameter("qmask", [P, 8, P], BF16, isOutput=False)
    ones_d = nc.declare_dram_parameter("ones_d", [P], F32, isOutput=False)
    ones_b = nc.declare_dram_parameter("ones_b", [P], BF16, isOutput=False)
    out = nc.declare_dram_parameter("out", [QL, D], F32, isOutput=True)

    with tile.TileContext(nc) as tc:
      with nc.allow_low_precision(reason="bf16 prob accum; fp32r broadcasts"):
        with (
            tc.tile_pool(name="pxt", bufs=16) as pxt,
            tc.tile_pool(name="pwp", bufs=16) as pwp,
            tc.tile_pool(name="pkv", bufs=1) as pkv,
            tc.tile_pool(name="pqa", bufs=1) as pqa,
            tc.tile_pool(name="pwk", bufs=2) as pwk,      # work tiles
            tc.tile_pool(name="ppt", bufs=2) as ppt,      # pT / rope tiles
            tc.tile_pool(name="pcst", bufs=1) as pcst,
            tc.tile_pool(name="pdram", bufs=1, space="DRAM") as pdram,
            tc.tile_pool(name="ps", bufs=1, space="PSUM") as ps,
        ):
            # 2-bank psum tile tags, cycled by phase parity
            PSA = ("sc0", "sc1")          # parity-0 pair of 2-bank tiles
            PSB = ("atp", "aux")          # parity-1 pair

            def ps2(tag):
                return ps.tile([P, 2, 512], F32, tag=tag, name=f"ps_{tag}")

            # ---- constants (gpsimd queue: off the critical DMA paths) ----
            cosq_sb = pcst.tile([DH, QL], F32, name="cosq_sb")
            sinq_sb = pcst.tile([DH, QL], BF16, name="sinq_sb")
            qmask_sb = pcst.tile([P, 8, P], BF16, name="qmask_sb")
            ones1 = pcst.tile([1, P], F32R, name="ones1")
            onesb128 = pcst.tile([P, 1], BF16, name="onesb128")
            nc.gpsimd.dma_start(out=cosq_sb, in_=cosq[:])
            nc.gpsimd.dma_start(out=sinq_sb, in_=sinq[:])
            nc.gpsimd.dma_start(out=qmask_sb, in_=qmask[:])
            nc.gpsimd.dma_start(
                out=ones1,
                in_=ones_d.rearrange("(o p) -> o p", o=1).bitcast(F32R))
            nc.gpsimd.dma_start(
                out=onesb128,
                in_=ones_b.rearrange("(p o) -> p o", o=1))

            kT_sb = pkv.tile([DH, HKV, T], BF16, name="kT_sb")
            v_sb = pkv.tile([P, NCH, HKV * DH], BF16, name="v_sb")
            kT_half = pkv.tile([DH, HKV, T // 2], BF16, name="kT_half")
            v_half = pkv.tile([P, NCH // 2, HKV * DH], BF16, name="v_half")
            ibk = pdram.tile([P, 4096], BF16, name="ibk")
            obk = pdram.tile([2, P, 4096], BF16, name="obk")
            ibv = pdram.tile([P, 4096], BF16, name="ibv")
            obv = pdram.tile([2, P, 4096], BF16, name="obv")

            def rope_apply(ps_raw, cos_ap, sin_ap, dest_ap):
                """dest = ps_raw*cos + shift(ps_raw)*sin' (sign folded in sin').

                The half-rotation is two partition-shifted SBUF->SBUF DMA
                copies of a raw evacuation (DMA cannot read PSUM); the psum
                bank frees once the raw copy + the cos-mul have read it.
                bf16 work tiles keep the DVE ops in 2x mode; the sin-mul
                runs on gpsimd to keep the DVE clear for dacc/normalize.
                """
                raw = ppt.tile([P, 512], BF16, tag="rraw", name="raw", bufs=2)
                nc.vector.tensor_copy(out=raw[:], in_=ps_raw)
                nc.vector.tensor_mul(out=dest_ap, in0=ps_raw, in1=cos_ap)
                tmp = ppt.tile([P, 512], BF16, tag="rtmp", name="tmp", bufs=2)
                nc.gpsimd.dma_start(out=tmp[0:64, :], in_=raw[64:128, :])
                nc.gpsimd.dma_start(out=tmp[64:128, :], in_=raw[0:64, :])
                t2 = pwk.tile([P, 512], BF16, tag="tsb", name="t2")
                nc.gpsimd.tensor_mul(out=t2[:], in0=tmp[:], in1=sin_ap)
                # the final add stays on gpsimd: on the vector FIFO it would
                # sit behind the t2 chain and block the next tile's
                # bank-freeing reads (raw/cos-mul), stalling the PE
                nc.gpsimd.tensor_add(out=dest_ap, in0=dest_ap, in1=t2[:])

            # ========== Phase A: K then V per tb, one xt stream ==========
            # Each core projects only its HALF of the time axis (the host
            # ships xT/cosk/sink pre-sliced); the halves are exchanged with
            # the pair partner via pair AllGathers below. Within a tb the
            # xt tiles are loaded once and reused by both the K matmuls
            # (PSA banks) and the V matmuls (PSB banks); the parity keeps
            # evacuations off the accumulation critical path.
            for tb in range(2):
                cosk_sb = pwk.tile([DH, 512], F32, tag="cosk", name="cosk_sb")
                sink_sb = pwk.tile([DH, 512], BF16, tag="sink", name="sink_sb")
                nc.gpsimd.dma_start(out=cosk_sb, in_=cosk[:, 512 * tb:512 * (tb + 1)])
                nc.gpsimd.dma_start(out=sink_sb, in_=sink[:, 512 * tb:512 * (tb + 1)])
                pstk = [ps2(PSA[0]), ps2(PSA[1])]
                psk = [pstk[kv // 2][:, kv % 2, :] for kv in range(HKV)]
                xt_tiles = []
                for c in range(NCH):
                    xt = pxt.tile([P, 512], BF16, tag="xt", name="xt")
                    nc.sync.dma_start(
                        out=xt,
                        in_=xT[P * c:P * (c + 1), 512 * tb:512 * (tb + 1)])
                    xt_tiles.append(xt)
                    wkc = pwp.tile([P, 512], BF16, tag="wst", name="wkc")
                    nc.scalar.dma_start(
                        out=wkc, in_=wkv[P * c:P * (c + 1), 0:512])
                    for kv in range(HKV):
                        nc.tensor.matmul(psk[kv],
                                         wkc[:, DH * kv:DH * (kv + 1)], xt[:],
                                         start=(c == 0), stop=(c == NCH - 1))
                pstv = [ps2(PSB[0]), ps2(PSB[1])]
                psv = [pstv[ks // 2][:, ks % 2, :] for ks in range(4)]
                for c in range(NCH):
                    wvc = pwp.tile([P, 512], BF16, tag="wst", name="wvc")
                    nc.scalar.dma_start(
                        out=wvc, in_=wkv[P * c:P * (c + 1), 512:1024])
                    for ks in range(4):
                        nc.tensor.matmul(psv[ks],
                                         xt_tiles[c][:, P * ks:P * (ks + 1)],
                                         wvc[:],
                                         start=(c == 0), stop=(c == NCH - 1))
                for kv in range(HKV):
                    rope_apply(psk[kv], cosk_sb[:], sink_sb[:],
                               kT_half[:, kv, 512 * tb:512 * (tb + 1)])
                for ks in range(4):
                    nc.scalar.copy(out=v_half[:, 4 * tb + ks, :], in_=psv[ks])
                # stagger the exchange uploads: this tb's pieces go to DRAM
                # while the next tb computes, so the collectives can fire
                # right at the end of phase A
                nc.sync.dma_start(
                    out=ibk.rearrange("p (k t) -> p k t",
                                      k=HKV)[:, :, 512 * tb:512 * (tb + 1)],
                    in_=kT_half[:, :, 512 * tb:512 * (tb + 1)])
                nc.sync.dma_start(
                    out=ibv.rearrange("p (c w) -> p c w",
                                      c=8)[:, 4 * tb:4 * (tb + 1), :],
                    in_=v_half[:, 4 * tb:4 * (tb + 1), :])

            # ---- exchange halves with the pair partner (cores 2b, 2b+1) ----
            # Two pair AllGathers (issue-only on the gpsimd queue); the
            # unpack DMAs are emitted after B(g0) so they never hold up the
            # xtq/wqc streams B needs first.
            nc.gpsimd.collective_compute(
                "AllGather", mybir.AluOpType.bypass,
                replica_groups=[[0, 1], [2, 3], [4, 5], [6, 7]],
                ins=[ibk.opt()], outs=[obk.opt()])
            nc.gpsimd.collective_compute(
                "AllGather", mybir.AluOpType.bypass,
                replica_groups=[[0, 1], [2, 3], [4, 5], [6, 7]],
                ins=[ibv.opt()], outs=[obv.opt()])

            # ============ Phases B+attn per query group g =================
            at_tiles = {}
            for g in range(2):
                # ---- Phase B: Q projection + RoPE for group g (quarters) ----
                q_tiles = {}
                xtq_tiles = []
                for quarter in range(4):
                    tags = PSA if quarter % 2 == 0 else PSB
                    pst = [ps2(tags[0]), ps2(tags[1])]
                    psq = [pst[j // 2][:, j % 2, :] for j in range(4)]
                    for c in range(NCH):
                        if quarter == 0:
                            xtq = pxt.tile([P, 512], BF16, tag="xt",
                                           name="xtq")
                            nc.sync.dma_start(
                                out=xtq,
                                in_=xTq[P * c:P * (c + 1),
                                        512 * g:512 * (g + 1)])
                            xtq_tiles.append(xtq)
                        wqc = pwp.tile([P, 512], BF16, tag="wst", name="wqc")
                        nc.scalar.dma_start(
                            out=wqc,
                            in_=wq[P * c:P * (c + 1),
                                   512 * quarter:512 * (quarter + 1)])
                        for j in range(4):
                            nc.tensor.matmul(psq[j],
                                             wqc[:, DH * j:DH * (j + 1)],
                                             xtq_tiles[c][:],
                                             start=(c == 0), stop=(c == NCH - 1))
                    for j in range(4):
                        head = 4 * quarter + j
                        qt = pqa.tile([P, 512], BF16, tag=f"q{head}", name="qt",
                                      bufs=1)
                        q_tiles[head] = qt
                        rope_apply(psq[j],
                                   cosq_sb[:, 512 * g:512 * (g + 1)],
                                   sinq_sb[:, 512 * g:512 * (g + 1)],
                                   qt[:])

                if g == 0:
                    # unpack the gathered halves (pair order == time order)
                    for hh in range(2):
                        nc.sync.dma_start(
                            out=kT_sb[:, :, 1024 * hh:1024 * (hh + 1)],
                            in_=obk[hh].rearrange("p (k t) -> p k t", k=HKV))
                        nc.sync.dma_start(
                            out=v_sb[:, 8 * hh:8 * (hh + 1), :],
                            in_=obv[hh].rearrange("p (c w) -> p c w", c=8))

                # ---- attention for group g: two lanes (even/odd heads) ----
                nfull = 8 * g
                nkc = nfull + 8
                nblk = nkc // 2
                pending_den = None
                for pair in range(H // 2):
                    heads = (2 * pair, 2 * pair + 1)
                    kv = heads[0] // (H // HKV)
                    at_ps = ps2("atp")
                    dacc = {}
                    for ln in range(2):
                        dacc[ln] = pwk.tile([P, 512], BF16, tag=f"dacc{ln}",
                                            name="dacc", bufs=1)

                    def blk_lo(blk):
                        # both kc in a block share lo (mi pairs 2m, 2m+1)
                        kc = 2 * blk
                        if kc < nfull:
                            return 0
                        return 128 * ((kc - nfull) // 2)

                    def emit_block(blk):
                        """Score matmuls for both lanes (one block = 2 kc)."""
                        lo = blk_lo(blk)
                        tiles = []
                        for ln in range(2):
                            qt = q_tiles[heads[ln]]
                            sc = ps2(f"sc{ln}")
                            for j in range(2):
                                kc = 2 * blk + j
                                nc.tensor.matmul(
                                    sc[:, j, lo:512],
                                    kT_sb[:, kv, P * kc:P * (kc + 1)],
                                    qt[:, lo:512], start=True, stop=True)
                            tiles.append(sc)
                        return tiles

                    sc_cur = emit_block(0)
                    # previous pair's denominator chain is emitted AFTER this
                    # pair's first scores so the PE queue never blocks on the
                    # dacc tail
                    if pending_den is not None:
                        pending_den()
                    for blk in range(nblk):
                        lo = blk_lo(blk)
                        sc_nxt = emit_block(blk + 1) if blk + 1 < nblk else None
                        for ln in range(2):
                            pT = ppt.tile([P, 2, 512], BF16, tag=f"pT{ln}",
                                          name="pT", bufs=2)
                            nc.scalar.activation(out=pT[:, :, lo:512],
                                                 in_=sc_cur[ln][:, :, lo:512],
                                                 func=AF.Exp)
                            for j in range(2):
                                kc = 2 * blk + j
                                mi = kc - nfull
                                if mi >= 0:
                                    # causal mask: multiply the diagonal
                                    # 128-query strip by a 0/1 mask (exp of
                                    # unmasked scores is bounded, ~e^5.5)
                                    nc.vector.tensor_mul(
                                        out=pT[:, j, lo:lo + P],
                                        in0=pT[:, j, lo:lo + P],
                                        in1=qmask_sb[:, mi, :])
                            for j in range(2):
                                kc = 2 * blk + j
                                nc.tensor.matmul(
                                    at_ps[:, ln, lo:512],
                                    v_sb[:, kc, DH * kv:DH * (kv + 1)],
                                    pT[:, j, lo:512],
                                    start=(kc == 0), stop=(kc == nkc - 1))
                            if blk == 0:
                                nc.vector.tensor_copy(out=dacc[ln][:],
                                                      in_=pT[:, 0, :])
                                nc.vector.tensor_add(out=dacc[ln][:],
                                                     in0=dacc[ln][:],
                                                     in1=pT[:, 1, :])
                            else:
                                for j in range(2):
                                    nc.vector.tensor_add(
                                        out=dacc[ln][:, lo:512],
                                        in0=dacc[ln][:, lo:512],
                                        in1=pT[:, j, lo:512])
                        sc_cur = sc_nxt

                    def make_den(dacc=dacc, at_ps=at_ps, heads=heads, g=g):
                        def den():
                            aux = ps2("aux")
                            for ln in range(2):
                                nc.tensor.matmul(aux[0:1, ln, :], onesb128[:],
                                                 dacc[ln][:],
                                                 start=True, stop=True)
                            for ln, head in enumerate(heads):
                                recip = ppt.tile([1, 512], F32, tag="recip",
                                                 name="recip", bufs=2)
                                nc.vector.reciprocal_approx_fast(
                                    out=recip[:], in_=aux[0:1, ln, :])
                                recip_r = ppt.tile([1, 512], F32R,
                                                   tag="recipr",
                                                   name="recip_r", bufs=2)
                                nc.vector.tensor_copy(out=recip_r[:],
                                                      in_=recip[:])
                                nc.tensor.matmul(aux[:, ln, :], ones1[:],
                                                 recip_r[:],
                                                 start=True, stop=True)
                                b_sb = pwk.tile([P, 512], F32, tag="eva",
                                                name="b_sb")
                                nc.vector.tensor_copy(out=b_sb[:],
                                                      in_=aux[:, ln, :])
                                at = pqa.tile([P, 512], BF16,
                                              tag=f"at{g}_{head}", name="at")
                                at_tiles[(g, head)] = at
                                nc.vector.tensor_mul(out=at[:],
                                                     in0=at_ps[:, ln, :],
                                                     in1=b_sb[:])
                        return den

                    pending_den = make_den()
                pending_den()

            # ================= Phase O: output projection ==================
            # wo chunks are loaded once per cg and reused for both halves;
            # psum parity alternates half0 -> PSA, half1 -> PSB.
            for cg in range(4):
                woc_tiles = []
                pst0 = [ps2(PSA[0]), ps2(PSA[1])]
                pso0 = [pst0[j // 2][:, j % 2, :] for j in range(4)]
                for c in range(NCH):
                    woc = pwp.tile([P, 512], BF16, tag="wst", name="woc")
                    nc.scalar.dma_start(
                        out=woc,
                        in_=wo[P * c:P * (c + 1), 512 * cg:512 * (cg + 1)])
                    woc_tiles.append(woc)
                    at = at_tiles[(0, c)]
                    for j in range(4):
                        nc.tensor.matmul(
                            pso0[j],
                            at[:, P * j:P * (j + 1)], woc[:],
                            start=(c == 0), stop=(c == NCH - 1))
                for j in range(4):
                    osb = pwk.tile([P, 512], F32, tag="eva", name="osb")
                    nc.scalar.copy(out=osb[:], in_=pso0[j])
                    nc.sync.dma_start(
                        out=out[P * j:P * (j + 1), 512 * cg:512 * (cg + 1)],
                        in_=osb[:])
                pst1 = [ps2(PSB[0]), ps2(PSB[1])]
                pso1 = [pst1[j // 2][:, j % 2, :] for j in range(4)]
                for c in range(NCH):
                    at = at_tiles[(1, c)]
                    for j in range(4):
                        nc.tensor.matmul(
                            pso1[j],
                            at[:, P * j:P * (j + 1)], woc_tiles[c][:],
                            start=(c == 0), stop=(c == NCH - 1))
                for j in range(4):
                    rs = 4 + j
                    osb = pwk.tile([P, 512], F32, tag="eva", name="osb2")
                    nc.vector.tensor_copy(out=osb[:], in_=pso1[j])
                    nc.sync.dma_start(
                        out=out[P * rs:P * (rs + 1),
                                512 * cg:512 * (cg + 1)],
                        in_=osb[:])

    nc.compile()
    return nc


def _host_prep(x, Wq, Wk, Wv, Wo):
    import ml_dtypes

    t = np.arange(T, dtype=np.float64)
    inv = 1.0 / (ROPE_BASE ** (np.arange(0, DH, 2, dtype=np.float64) / DH))
    ang = np.concatenate([np.outer(t, inv), np.outer(t, inv)], axis=1)  # [T,DH]
    cos = np.cos(ang).T.astype(np.float32).copy()   # [DH, T]
    sin = np.sin(ang).T.astype(np.float32).copy()
    # sign-folded sin for the DMA-shift RoPE: rows 0..63 get -sin (they
    # multiply the shifted-down second half), rows 64..127 get +sin.
    sin2 = sin.copy()
    sin2[:DH // 2] *= -1.0
    scale = np.float32(1.0 / np.sqrt(DH))

    # multiplicative causal mask: 1 = keep, 0 = drop
    tri = (np.arange(P)[:, None] <= np.arange(P)[None, :]).astype(np.float32)
    qmask = np.zeros((2, 8, P, P), np.float32)
    for h in range(2):
        for i in range(8):
            if i % 2 == 0:
                qmask[h, i] = tri if h == 0 else 1.0
            else:
                qmask[h, i] = 0.0 if h == 0 else tri

    qrows = [np.concatenate([np.arange(P * (2 * s + h), P * (2 * s + h) + P)
                             for s in range(8)]) for h in range(2)]
    ones = np.ones(P, np.float32)

    Wo_bf16 = Wo.astype(ml_dtypes.bfloat16)
    Wq_bf16 = np.ascontiguousarray(Wq.astype(ml_dtypes.bfloat16))
    Wkv_bf16 = np.ascontiguousarray(
        np.concatenate([Wk, Wv], axis=1).astype(ml_dtypes.bfloat16))

    in_maps = []
    for core in range(NC_COUNT):
        b, h = core // 2, core % 2
        xTb = np.ascontiguousarray(x[b].T).astype(ml_dtypes.bfloat16)  # [D, T]
        in_maps.append({
            "xT": np.ascontiguousarray(xTb[:, 1024 * h:1024 * (h + 1)]),
            "xTq": np.ascontiguousarray(xTb[:, qrows[h]]),
            "wq": Wq_bf16,
            "wkv": Wkv_bf16,
            "wo": Wo_bf16,
            "cosq": np.ascontiguousarray(cos[:, qrows[h]] * scale),
            "sinq": np.ascontiguousarray(
                (sin2[:, qrows[h]] * scale).astype(ml_dtypes.bfloat16)),
            "cosk": np.ascontiguousarray(cos[:, 1024 * h:1024 * (h + 1)]),
            "sink": np.ascontiguousarray(
                sin2[:, 1024 * h:1024 * (h + 1)].astype(ml_dtypes.bfloat16)),
            "qmask": np.ascontiguousarray(
                qmask[h].transpose(1, 0, 2).astype(ml_dtypes.bfloat16)),
            "ones_d": ones,
            "ones_b": ones.astype(ml_dtypes.bfloat16),
        })
    return in_maps, qrows


def kernel(x, Wq, Wk, Wv, Wo):
    x = np.asarray(x, np.float32)
    Wq = np.ascontiguousarray(np.asarray(Wq, np.float32))
    Wk = np.ascontiguousarray(np.asarray(Wk, np.float32))
    Wv = np.ascontiguousarray(np.asarray(Wv, np.float32))
    Wo = np.ascontiguousarray(np.asarray(Wo, np.float32))

    if "nc" not in _CACHE:
        _CACHE["nc"] = _build()
    nc = _CACHE["nc"]

    in_maps, qrows = _host_prep(x, Wq, Wk, Wv, Wo)
    _CACHE["in_maps"] = in_maps

    r = run_bass_kernel_spmd(nc, in_maps, list(range(NC_COUNT)))
    _CACHE["results"] = r

    out = np.empty((B, T, D), np.float32)
    for core in range(NC_COUNT):
        b, h = core // 2, core % 2
        out[b, qrows[h], :] = r.results[core]["out"]
    return out
